# revision 22
# baseline (speedup 1.0000x reference)
"""EntropyGuidedAttention on 8 Trainium2 NeuronCores.

Sharding: data-parallel over batch (2) x tensor-parallel over heads (16/4=4
per core).  Core c handles batch c//4 and heads [4*(c%4), 4*(c%4)+4).
qkv is column-parallel, out_proj row-parallel; the per-batch sum over the
4 head-group partials (an AllReduce in classic TP) is done on the host as
part of unsharding, along with + b_out.

Device math per core (weights pre-folded on host):
  xn   = (x - mu) * rsqrt(var + 1e-6)                  (ln_g/ln_b folded into W)
  qT,kT = Wq'/Wk' blocks @ xn^T   (Wq' includes scale/TEMP = 1.25)
  v     = xn @ Wv'^T ; gate = clip(sigmoid(xn @ we' + be'), .1, 2); v' = (v+vb)*gate
  St    = kT^T q (scores transposed, [k, q] layout), Pt = exp(St) * causal
  numT  = sum_kt v'^T @ Pt ; Z = ones^T @ quadfold(Pt)  (Pt quad-sums on DVE)
  OT    = numT * (1/Z broadcast via selector matmul)
  out_p = sum_p OT_p^T @ Wo_p     (Wo includes the 0.1 output scale)

Engine budget: the Act engine uses ONLY {Exp, Ln, Identity, Copy} so a single
activation-table set is loaded once (sigmoid is computed as exp + DVE ops,
rsqrt as exp(-0.5*ln(v+eps))).  Weights are pre-rearranged on the host so all
weight DMAs are contiguous per partition.  The out-projection is folded into
the attention qc loop so the output DMA overlaps compute.
"""
import contextlib

import numpy as np

import concourse.bacc as bacc
import concourse.tile as tile
from concourse import mybir
from concourse.bass_utils import run_bass_kernel_spmd

F32 = mybir.dt.float32
F32R = mybir.dt.float32r
BF16 = mybir.dt.bfloat16
AF = mybir.ActivationFunctionType
ALU = mybir.AluOpType

H, NH, HD = 1024, 16, 64
B, S = 2, 2048
NCORES = 8
HPC = 4            # heads per core
NPAIR = 2          # head pairs per core
ST = S // 128      # 16 s-tiles
KC = H // 128      # 8 contraction chunks
QC = S // 512      # 4 q chunks of 512


def _build_nc():
    nc = bacc.Bacc("TRN2", target_bir_lowering=False, debug=False,
                   num_devices=NCORES)

    x_d = nc.dram_tensor("x", [S, H], F32, kind="ExternalInput")
    wqk_d = nc.dram_tensor("wqkT", [128, KC * 512], F32, kind="ExternalInput")
    wvg_d = nc.dram_tensor("wvg", [128, KC * 258], F32, kind="ExternalInput")
    wo_d = nc.dram_tensor("wo", [128, 2 * H], F32, kind="ExternalInput")
    qkb_d = nc.dram_tensor("qkb", [128, 4], F32, kind="ExternalInput")
    vb_d = nc.dram_tensor("vb", [256], F32, kind="ExternalInput")
    negentb_d = nc.dram_tensor("negentb", [1], F32, kind="ExternalInput")
    ident_d = nc.dram_tensor("ident", [128, 128], F32, kind="ExternalInput")
    umask_d = nc.dram_tensor("umask", [128, 128], F32, kind="ExternalInput")
    sel_d = nc.dram_tensor("sel", [128, 256], F32, kind="ExternalInput")
    out_d = nc.dram_tensor("out_part", [S, H], F32, kind="ExternalOutput")

    with tile.TileContext(nc) as tc, contextlib.ExitStack() as ctx:
        consts = ctx.enter_context(tc.tile_pool(name="consts", bufs=1))
        qk_pool = ctx.enter_context(tc.tile_pool(name="qk", bufs=1))
        vg_pool = ctx.enter_context(tc.tile_pool(name="vg", bufs=1))
        ot_pool = ctx.enter_context(tc.tile_pool(name="ot", bufs=1))

        # ---- constants: x tiles get the sync queue to themselves so the
        # first tile lands ASAP; weights + ident ride the gpsimd queue,
        # small consts the scalar queue; wo is deferred to phase 5 ----
        identb = consts.tile([128, 128], F32R)
        nc.gpsimd.dma_start(out=identb, in_=ident_d[:, :].bitcast(F32R))
        # wqk/wvg tiles are allocated here but their DMAs are issued inside
        # phase 1-4, gated on the first x tiles' arrival, so the 3MB weight
        # stream does not starve the startup-critical x DMAs of HBM bandwidth
        wqk = consts.tile([128, KC, 512], F32R)
        wvg = consts.tile([128, KC, 258], F32R)
        umask = consts.tile([128, NPAIR, 128], BF16)
        nc.gpsimd.dma_start(
            out=umask, in_=umask_d.rearrange("p (u m) -> p u m", u=1)
            .to_broadcast([128, NPAIR, 128]))
        sel = consts.tile([128, 256], F32R)
        nc.scalar.dma_start(out=sel, in_=sel_d[:, :].bitcast(F32R))
        qkb = consts.tile([128, 4], F32)
        nc.scalar.dma_start(out=qkb, in_=qkb_d[:, :])
        vb = consts.tile([128, 256], F32)
        nc.scalar.dma_start(out=vb, in_=vb_d[None, :].to_broadcast([128, 256]))
        negentb = consts.tile([128, 1], F32)
        nc.scalar.dma_start(out=negentb,
                            in_=negentb_d[None, :].to_broadcast([128, 1]))
        ones32 = consts.tile([128, 32], BF16)
        nc.vector.memset(ones32, 1.0)

        qk_big = qk_pool.tile([128, 4, S], F32R)      # qp0 qp1 kp0 kp1
        vg_big = vg_pool.tile([128, ST, 256], BF16)   # gated v, s-tile major
        ot_big = ot_pool.tile([128, NPAIR, S], F32R)  # O^T (pair, d) x q

        # ---- phases 1-4, interleaved by groups of 4 s-tiles:
        # layernorm + transpose -> xnT (per-group ring), then the QKV-T chunk
        # and V+gate tiles that only need those xnT columns ----
        with tc.tile_pool(name="ln", bufs=8) as ln_pool, \
             tc.tile_pool(name="xn", bufs=4) as xn_pool, \
             tc.tile_pool(name="stats", bufs=4) as stats_pool, \
             tc.tile_pool(name="xnt", bufs=2) as xnt_pool, \
             tc.tile_pool(name="pst", bufs=4, space="PSUM") as pst, \
             tc.tile_pool(name="psq", bufs=2, space="PSUM") as psq, \
             tc.tile_pool(name="psv", bufs=2, space="PSUM") as psv:

            RSQRT_MAGIC = 0x5F3759DF
            I32 = mybir.dt.int32

            def rsqrt_chain(var_ap, vpe, yr, tr_):
                # rstd = rsqrt(var + eps) on DVE: quake-style int seed + two
                # Newton steps (keeps the Act engine on its one exp table set)
                nc.vector.tensor_scalar(out=vpe, in0=var_ap,
                                        scalar1=1e-6, scalar2=None,
                                        op0=ALU.add)
                nc.vector.tensor_scalar(out=yr.bitcast(I32),
                                        in0=vpe.bitcast(I32),
                                        scalar1=1, scalar2=-1,
                                        op0=ALU.logical_shift_right,
                                        op1=ALU.bitwise_xor)
                nc.vector.tensor_scalar(out=yr.bitcast(I32),
                                        in0=yr.bitcast(I32),
                                        scalar1=RSQRT_MAGIC + 1, scalar2=None,
                                        op0=ALU.add)
                for _ in range(2):
                    nc.vector.tensor_mul(tr_, yr, yr)
                    nc.vector.tensor_mul(tr_, tr_, vpe)
                    nc.vector.tensor_scalar(out=tr_, in0=tr_,
                                            scalar1=-0.5, scalar2=1.5,
                                            op0=ALU.mult, op1=ALU.add)
                    nc.vector.tensor_mul(yr, yr, tr_)

            def emit_stats(g, per_tile):
                """DMA + bn stats + rsqrt for group g (DVE + DMA only).
                per_tile (cold start): each tile gets its OWN stats/scratch
                tiles and an Act-engine xn, so tile j's transposes depend on
                exactly tile j's 13 DVE ops — no false tile-sharing with
                later (DMA-starved) tiles, no DVE-scheduler coupling."""
                xts = []
                for j in range(4):
                    st = 4 * g + j
                    xt = ln_pool.tile([128, H], F32, tag="x")
                    nc.sync.dma_start(out=xt,
                                      in_=x_d[st * 128:(st + 1) * 128, :])
                    xts.append(xt)
                if per_tile:
                    xns = []
                    mvg = stats_pool.tile([128, 4, 2], F32, tag="mv")
                    yrg = stats_pool.tile([128, 4], F32, tag="yr")
                    for j in range(4):
                        stats = stats_pool.tile([128, 2, 6], F32,
                                                tag=f"bn{j}")
                        nc.vector.bn_stats(out=stats[:, 0, :],
                                           in_=xts[j][:, 0:512])
                        nc.vector.bn_stats(out=stats[:, 1, :],
                                           in_=xts[j][:, 512:1024])
                        mvj = stats_pool.tile([128, 2], F32, tag=f"mvj{j}")
                        nc.vector.bn_aggr(out=mvj, in_=stats)
                        scr = stats_pool.tile([128, 4], F32, tag=f"scr{j}")
                        rsqrt_chain(mvj[:, 1:2], scr[:, 0:1], scr[:, 1:2],
                                    scr[:, 2:3])
                        nc.vector.tensor_scalar(out=scr[:, 3:4],
                                                in0=mvj[:, 0:1],
                                                scalar1=scr[:, 1:2],
                                                scalar2=-1.0,
                                                op0=ALU.mult, op1=ALU.mult)
                        xn = xn_pool.tile([128, H], F32R, tag="xn")
                        nc.scalar.activation(out=xn, in_=xts[j],
                                             func=AF.Identity,
                                             bias=scr[:, 3:4],
                                             scale=scr[:, 1:2])
                        xns.append(xn)
                    return xts, mvg, yrg, xns
                mvg = stats_pool.tile([128, 4, 2], F32, tag="mv")
                yr = stats_pool.tile([128, 4], F32, tag="yr")
                vpe = stats_pool.tile([128, 4], F32, tag="vpe")
                tr_ = stats_pool.tile([128, 4], F32, tag="tr")
                for j in range(4):
                    stats = stats_pool.tile([128, 2, 6], F32, tag="bn")
                    nc.vector.bn_stats(out=stats[:, 0, :], in_=xts[j][:, 0:512])
                    nc.vector.bn_stats(out=stats[:, 1, :],
                                       in_=xts[j][:, 512:1024])
                    nc.vector.bn_aggr(out=mvg[:, j, :], in_=stats)
                rsqrt_chain(mvg[:, :, 1], vpe[:, :], yr[:, :], tr_[:, :])
                return xts, mvg, yr, None

            def emit_compute(g, xts, mvg, yr, xns=None):
                """xn + transposes + QKV + V/gate for group g; emits the next
                group's stats on the DVE queue right after this group's xn
                ops so the rstd chain is ready one group ahead."""
                xnt = xnt_pool.tile([128, KC, 512], F32R, tag="xnt")
                if xns is None:
                    xns = []
                    for j in range(4):
                        xn = xn_pool.tile([128, H], F32R, tag="xn")
                        nc.vector.tensor_scalar(out=xn, in0=xts[j],
                                                scalar1=mvg[:, j, 0:1],
                                                scalar2=yr[:, j:j + 1],
                                                op0=ALU.subtract, op1=ALU.mult)
                        xns.append(xn)
                for j in range(4):
                    # transpose 8 h-chunks; 4 per psum bank; psum->sbuf
                    # copies on Act (DVE is the busier engine here)
                    for half in range(2):
                        ptr = pst.tile([128, 4, 128], F32R, tag="tr")
                        for c4 in range(4):
                            c = half * 4 + c4
                            nc.tensor.transpose(ptr[:, c4, :],
                                                xns[j][:, c * 128:(c + 1) * 128],
                                                identb)
                        nc.scalar.copy(xnt[:, half * 4:half * 4 + 4,
                                           j * 128:(j + 1) * 128], ptr)
                # QKV-T for this 512-wide chunk of S (bias add on Act)
                for mb in range(4):
                    pq = psq.tile([128, 512], F32, tag="q")
                    for c in range(KC):
                        nc.tensor.matmul(pq[:, :],
                                         wqk[:, c, mb * 128:(mb + 1) * 128],
                                         xnt[:, c, :],
                                         start=(c == 0), stop=(c == KC - 1))
                    nc.scalar.activation(
                        out=qk_big[:, mb, g * 512:(g + 1) * 512], in_=pq[:, :],
                        func=AF.Identity, bias=qkb[:, mb:mb + 1], scale=1.0)

                # V + entropy gate for these 4 s-tiles
                for j in range(4):
                    st = 4 * g + j
                    pv = psv.tile([128, 258], F32, tag="v")
                    for c in range(KC):
                        nc.tensor.matmul(pv[:, :],
                                         xnt[:, c, j * 128:(j + 1) * 128],
                                         wvg[:, c, :],
                                         start=(c == 0), stop=(c == KC - 1))
                    # gate = max(0.1, 1/(1+exp(-(y+bent)))) (sigmoid<1 so the
                    # 2.0 upper clip never binds); exp on Act, rest on DVE
                    gex = stats_pool.tile([128, 1], F32, tag="gex")
                    nc.scalar.activation(out=gex, in_=pv[:, 256:257],
                                         func=AF.Exp, bias=negentb, scale=-1.0)
                    gden = stats_pool.tile([128, 1], F32, tag="gden")
                    nc.vector.tensor_scalar(out=gden, in0=gex, scalar1=1.0,
                                            scalar2=None, op0=ALU.add)
                    gcol = stats_pool.tile([128, 1], F32, tag="gate")
                    nc.vector.reciprocal_approx_fast(out=gcol, in_=gden)
                    nc.vector.tensor_scalar(out=gcol, in0=gcol, scalar1=0.1,
                                            scalar2=None, op0=ALU.max)
                    vtmp = ln_pool.tile([128, 256], F32, tag="vtmp")
                    nc.vector.tensor_add(vtmp, pv[:, 0:256], vb)
                    nc.vector.tensor_scalar(out=vg_big[:, st, :], in0=vtmp,
                                            scalar1=gcol, scalar2=None,
                                            op0=ALU.mult)
                # next group's stats go at the END of this group's DVE work:
                # the in-order DVE queue (and walrus's local reordering) can
                # then never park a DMA-gated bn_stats ahead of ops the PE
                # is waiting on
                return emit_stats(g + 1, per_tile=False) if g + 1 < QC else None

            cur = emit_stats(0, per_tile=True)
            # gate the big weight DMAs on x tile 2's arrival (dummy Pool-
            # engine read) so they don't contend with the startup x stream
            wsync = stats_pool.tile([128, 1], F32, tag="wsync")
            nc.gpsimd.tensor_copy(wsync, cur[0][2][:, 0:1])
            nc.gpsimd.dma_start(
                out=wqk, in_=wqk_d.rearrange("p (c m) -> p c m", c=KC).bitcast(F32R))
            nc.gpsimd.dma_start(
                out=wvg, in_=wvg_d.rearrange("p (c m) -> p c m", c=KC).bitcast(F32R))
            for g in range(QC):
                cur = emit_compute(g, *cur)

        # ---- phase 5: attention + out-projection, per 512-wide q chunk.
        # St/exp are split per head-pair so the Act engine pipeline never
        # blocks the PE queue; Pt quads are pre-summed on DVE so the Z
        # (softmax denominator) matmul runs once per 4 k-tiles ----
        with tc.tile_pool(name="wop", bufs=1) as wo_pool, \
             tc.tile_pool(name="pt", bufs=3) as pt_pool, \
             tc.tile_pool(name="zf", bufs=2) as zf_pool, \
             tc.tile_pool(name="zw", bufs=2) as zw_pool, \
             tc.tile_pool(name="ost", bufs=4) as ost_pool, \
             tc.tile_pool(name="ps_st", bufs=1, space="PSUM") as ps_st, \
             tc.tile_pool(name="ps_pv", bufs=1, space="PSUM") as ps_pv, \
             tc.tile_pool(name="ps_zx", bufs=2, space="PSUM") as ps_zx:

            wo = wo_pool.tile([128, 2, H], F32R)
            nc.gpsimd.dma_start(
                out=wo, in_=wo_d.rearrange("p (c m) -> p c m", c=2).bitcast(F32R))

            def emit_pv(pvp, pts, qc, kt, nkt):
                # columns left of the causal diagonal are never touched:
                # the matmuls accumulate only into the valid [off:] range.
                first, last = kt == 0, kt == nkt - 1
                off = max(kt * 128 - qc * 512, 0)
                for p in range(NPAIR):
                    for a in range(2):
                        h = 2 * p + a
                        nc.tensor.matmul(
                            pvp[p][64 * a:64 * a + 64, off:],
                            vg_big[:, kt, h * 64:(h + 1) * 64],
                            pts[p][:, a, off:],
                            start=first, stop=last,
                            tile_position=(0, 64 * a))

            for qc in range(QC):
                nkt = 4 * qc + 4
                pvp = [ps_pv.tile([128, 512], F32, name=f"pv{p}_{qc}",
                                  tag=f"pv{p}")
                       for p in range(NPAIR)]
                pz = ps_zx.tile([128, 512], F32, name=f"pz_{qc}", tag="zx")
                prev = None
                first_pt = None
                zf = None
                for kt in range(nkt):
                    off = max(kt * 128 - qc * 512, 0)
                    cur = []
                    for p in range(NPAIR):
                        stp = ps_st.tile([128, 2, 512], F32,
                                         name=f"st_{qc}_{kt}_{p}", tag=f"s{p}")
                        for a in range(2):
                            nc.tensor.matmul(
                                stp[:, a, off:],
                                qk_big[64 * a:64 * a + 64, 2 + p,
                                       kt * 128:(kt + 1) * 128],
                                qk_big[64 * a:64 * a + 64, p,
                                       qc * 512 + off:(qc + 1) * 512],
                                start=True, stop=True,
                                tile_position=(64 * a, 0))
                        pt = pt_pool.tile([128, 2, 512], BF16,
                                          name=f"pt_{qc}_{kt}_{p}",
                                          tag=f"pt{p}")
                        nc.scalar.activation(out=pt[:, :, off:],
                                             in_=stp[:, :, off:], func=AF.Exp)
                        if kt * 128 >= qc * 512:   # diagonal k-tile
                            nc.vector.tensor_mul(pt[:, :, off:off + 128],
                                                 pt[:, :, off:off + 128],
                                                 umask)
                        cur.append(pt)
                    if prev is not None:
                        emit_pv(pvp, prev[1], qc, prev[0], nkt)
                    # Z fold on DVE: running bf16 sum of Pt over the whole q
                    # chunk; sliced adds keep every read inside exp-written
                    # columns (unwritten columns are causally-zero anyway)
                    if kt == 0:
                        first_pt = cur
                    elif kt == 1:
                        zf = [zf_pool.tile([128, 2, 512], BF16,
                                           name=f"zf_{qc}_{p}",
                                           tag=f"zf{p}")
                              for p in range(NPAIR)]
                        for p in range(NPAIR):
                            if off > 0:
                                nc.vector.tensor_copy(
                                    zf[p][:, :, 0:off], first_pt[p][:, :, 0:off])
                            nc.vector.tensor_add(zf[p][:, :, off:],
                                                 first_pt[p][:, :, off:],
                                                 cur[p][:, :, off:])
                    else:
                        for p in range(NPAIR):
                            nc.vector.tensor_add(zf[p][:, :, off:],
                                                 zf[p][:, :, off:],
                                                 cur[p][:, :, off:])
                    prev = (kt, cur)
                # softmax denominator: one 4-head Z matmul per q chunk
                for h in range(HPC):
                    nc.tensor.matmul(pz[32 * h:32 * h + 32, :],
                                     ones32,
                                     zf[h // 2][:, h % 2, :],
                                     start=True, stop=True,
                                     tile_position=(0, 32 * h))
                emit_pv(pvp, prev[1], qc, prev[0], nkt)

                # normalize: OT = numT * (1/Z), Z broadcast by selector matmul
                zsb = zw_pool.tile([128, 512], F32R, tag="zsb")
                nc.vector.tensor_copy(zsb, pz)
                for p in range(NPAIR):
                    pzb = ps_zx.tile([128, 512], F32, name=f"zb_{qc}_{p}",
                                     tag="zx")
                    nc.tensor.matmul(pzb[:, :],
                                     sel[:, p * 128:(p + 1) * 128],
                                     zsb[:, :], start=True, stop=True)
                    rzb = zw_pool.tile([128, 512], F32, tag="rzb")
                    nc.vector.reciprocal_approx_fast(out=rzb, in_=pzb)
                    nc.vector.tensor_mul(
                        ot_big[:, p, qc * 512:(qc + 1) * 512], pvp[p], rzb)

                # out projection for this q chunk's 4 s-tiles, psum -> DRAM
                for j in range(4):
                    st = 4 * qc + j
                    for n in range(2):
                        po = ps_zx.tile([128, 512], F32,
                                        name=f"op_{st}_{n}", tag="zx")
                        for p in range(NPAIR):
                            nc.tensor.matmul(
                                po[:, :],
                                ot_big[:, p, st * 128:(st + 1) * 128],
                                wo[:, p, n * 512:(n + 1) * 512],
                                start=(p == 0), stop=(p == NPAIR - 1))
                        ob = ost_pool.tile([128, 512], F32, tag="ob")
                        if (st + n) % 2 == 0:
                            nc.vector.tensor_copy(ob, po)
                        else:
                            nc.scalar.copy(ob, po)
                        nc.sync.dma_start(
                            out=out_d[st * 128:(st + 1) * 128,
                                      n * 512:(n + 1) * 512],
                            in_=ob[:, :])

    nc.compile()
    return nc


_NC = None


def _get_nc():
    global _NC
    if _NC is None:
        _NC = _build_nc()
    return _NC


def _in_maps(inputs):
    x = np.ascontiguousarray(np.asarray(inputs["x"], np.float32))
    ln_g = np.asarray(inputs["ln_g"], np.float32)
    ln_b = np.asarray(inputs["ln_b"], np.float32)
    w_qkv = np.asarray(inputs["w_qkv"], np.float32)
    b_qkv = np.asarray(inputs["b_qkv"], np.float32)
    w_ent = np.asarray(inputs["w_ent"], np.float32)
    b_ent = np.asarray(inputs["b_ent"], np.float32)

    qmul = np.float32((1.0 / np.sqrt(np.float32(HD))) / 0.1)

    wq = w_qkv[:H] * ln_g[None, :]
    wk = w_qkv[H:2 * H] * ln_g[None, :]
    wv = w_qkv[2 * H:] * ln_g[None, :]
    bq = (b_qkv[:H] + wq @ ln_b) * qmul
    bk = b_qkv[H:2 * H] + wk @ ln_b
    bv = b_qkv[2 * H:] + wv @ ln_b
    wq = wq * qmul
    went = (w_ent * ln_g[None, :])[0]
    bent = np.float32(b_ent[0] + w_ent[0] @ ln_b)
    w_out = np.asarray(inputs["w_out"], np.float32)

    ident = np.eye(128, dtype=np.float32)
    umask = np.triu(np.ones((128, 128), np.float32))
    sel = np.zeros((128, 256), np.float32)
    for p in range(NPAIR):
        sel[32 * (2 * p), p * 128:p * 128 + 64] = 1.0
        sel[32 * (2 * p + 1), p * 128 + 64:p * 128 + 128] = 1.0

    in_maps = []
    for c in range(NCORES):
        b, g = divmod(c, NCORES // B)
        r = slice(g * HPC * HD, (g + 1) * HPC * HD)
        # host-side re-layout to the device [partition, chunk, col] order so
        # every weight DMA is contiguous per partition
        wqkT = np.concatenate([wq[r], wk[r]], axis=0).T          # [H, 512]
        wqk_a = np.ascontiguousarray(
            wqkT.reshape(KC, 128, 512).transpose(1, 0, 2)).reshape(128, -1)
        wvgT = np.concatenate([wv[r], went[None, :],
                               np.zeros((1, H), np.float32)], axis=0).T
        wvg_a = np.ascontiguousarray(
            wvgT.reshape(KC, 128, 258).transpose(1, 0, 2)).reshape(128, -1)
        woT = (0.1 * w_out[:, r]).T                              # [256, H]
        wo_a = np.ascontiguousarray(
            woT.reshape(2, 128, H).transpose(1, 0, 2)).reshape(128, -1)
        qkb_a = np.ascontiguousarray(
            np.concatenate([bq[r], bk[r]]).reshape(4, 128).T)    # [128, 4]
        in_maps.append({
            "x": x[b], "wqkT": wqk_a, "wvg": wvg_a, "wo": wo_a,
            "qkb": qkb_a, "vb": np.ascontiguousarray(bv[r]),
            "negentb": np.array([-bent], np.float32),
            "ident": ident, "umask": umask, "sel": sel,
        })
    return in_maps


def _unshard(inputs, results):
    b_out = np.asarray(inputs["b_out"], np.float32)
    outs = []
    for b in range(B):
        g0 = b * (NCORES // B)
        acc = results[g0]["out_part"].astype(np.float32)
        for g in range(g0 + 1, g0 + NCORES // B):
            acc = acc + results[g]["out_part"]
        outs.append(acc + 0.1 * b_out[None, :])
    return np.stack(outs)


def run(inputs, **kw):
    nc = _get_nc()
    res = run_bass_kernel_spmd(nc, _in_maps(inputs),
                               core_ids=list(range(NCORES)), **kw)
    return _unshard(inputs, res.results), res


def kernel(**inputs) -> np.ndarray:
    out, _ = run(inputs)
    return out
